# revision 1
# baseline (speedup 1.0000x reference)
"""Trainium2 Bass kernel for 1D multi-scale deformable attention.

Self-contained: builds the Bass/Tile program, shards the full inputs
data-parallel over N across 8 NeuronCores, runs via run_bass_kernel_spmd,
and returns the full (N, LQ, 256) output.

Algorithm per core (one batch element):
  value = vin @ W_val.T + b_val            -> padded natural layout (T', 256)
  offs' = q @ (W_off/T).T + b_off/T        -> x = ref + offs' ; ix = x*T - 0.5
  attn  = softmax(q @ W_attn.T + b_attn)   per (q, m) over 16 (l,p)
  bilinear + zero padding == sum_t relu(1 - |ix - t|) * V[t], t in [0, T)
  per (q,l): one all-head window, base = min over (m,p) of needlo (clamped),
  static width W_l; indirect-DMA gathers W_l full 1KB value rows per query
  u[m,j] = sum_p attn * relu(1 - |ix_p - (base+j)|)
  out[q, m*32+d] = sum_{l,j} u * G
"""
import os
import numpy as np
from contextlib import ExitStack

import concourse.bass as bass
import concourse.bacc as bacc
import concourse.tile as tile
from concourse import mybir
from concourse.masks import make_identity
from concourse.bass_utils import run_bass_kernel_spmd

f32 = mybir.dt.float32
i32 = mybir.dt.int32
ALU = mybir.AluOpType
ACT = mybir.ActivationFunctionType

# static problem config
LENS = (2048, 1024, 512, 256)
N, LQ, DM = 8, 2048, 256
M, L, P, DH = 8, 4, 4, 32
S = sum(LENS)                      # 3840
WCONF = (8, 10, 8, 10)             # per-level all-head window rows (l3 padded to pair l1)
PAD = 12                           # zero rows after each level (>= max(W)-1)
LSTARTP = []
_s = 0
for _T in LENS:
    LSTARTP.append(_s)
    _s += _T + PAD
TPR = _s                           # 3888 padded rows total
NQT = LQ // 128                    # 16 query tiles
NVT = S // 128                     # 30 value tiles
BIG = 100000.0

# consts layout (one row, broadcast to 128 partitions at load)
C_TVEC = 0           # 128: T_l per c (c = m*16+l*4+p)
C_TM1L = 128         # 4:  T_l - 1
C_LST = 132          # 4:  LSTARTP[l]
C_JROW = 136         # 16: j = 0..15
C_NEG1 = 152         # 1: -1.0
CW = 153


def _ap(base, dims, extra_offset=0):
    """Custom strided AP derived from a 2D (128, F) contiguous tile AP."""
    return bass.AP(
        tensor=base.tensor,
        offset=base.offset + extra_offset,
        ap=[list(base.ap[0])] + [[s, c] for s, c in dims],
    )


def build_program():
    nc = bacc.Bacc("TRN2", target_bir_lowering=False, debug=False)

    q_d = nc.dram_tensor("q", [LQ, DM], f32, kind="ExternalInput")
    ref_d = nc.dram_tensor("ref", [LQ, L], f32, kind="ExternalInput")
    vin_d = nc.dram_tensor("vin", [S, DM], f32, kind="ExternalInput")
    wv_d = nc.dram_tensor("wv", [DM + 1, DM], f32, kind="ExternalInput")
    wof_d = nc.dram_tensor("wof", [DM + 1, M * L * P], f32, kind="ExternalInput")
    wat_d = nc.dram_tensor("wat", [DM + 1, M * L * P], f32, kind="ExternalInput")
    consts_d = nc.dram_tensor("consts", [1, CW], f32, kind="ExternalInput")
    out_d = nc.dram_tensor("out", [LQ, DM], f32, kind="ExternalOutput")

    with tile.TileContext(nc) as tc, ExitStack() as ctx:
        singles = ctx.enter_context(tc.tile_pool(name="singles", bufs=1))
        dram = ctx.enter_context(tc.tile_pool(name="dram", bufs=1, space="DRAM"))
        vpool = ctx.enter_context(tc.tile_pool(name="vpool", bufs=3))
        psum = ctx.enter_context(tc.tile_pool(name="psum", bufs=2, space="PSUM"))
        qpool = ctx.enter_context(tc.tile_pool(name="qpool", bufs=2))
        gpool = ctx.enter_context(tc.tile_pool(name="gpool", bufs=2))
        spool = ctx.enter_context(tc.tile_pool(name="spool", bufs=2))

        # ---- constants / weights (loaded once)
        ident = singles.tile([128, 128], f32)
        make_identity(nc, ident[:])
        ones_row = singles.tile([1, 128], f32)
        nc.vector.memset(ones_row[:], 1.0)
        consts = singles.tile([128, CW], f32)
        nc.sync.dma_start(
            out=consts[:],
            in_=bass.AP(tensor=consts_d[:].tensor, offset=0,
                        ap=[[0, 128], [1, CW]]),
        )
        wv0 = singles.tile([128, DM], f32)
        wv1 = singles.tile([128, DM], f32)
        wvb = singles.tile([1, DM], f32)
        nc.sync.dma_start(out=wv0[:], in_=wv_d[0:128, :])
        nc.sync.dma_start(out=wv1[:], in_=wv_d[128:256, :])
        nc.sync.dma_start(out=wvb[:], in_=wv_d[256:257, :])
        wof0 = singles.tile([128, 128], f32)
        wof1 = singles.tile([128, 128], f32)
        wofb = singles.tile([1, 128], f32)
        nc.sync.dma_start(out=wof0[:], in_=wof_d[0:128, :])
        nc.sync.dma_start(out=wof1[:], in_=wof_d[128:256, :])
        nc.sync.dma_start(out=wofb[:], in_=wof_d[256:257, :])
        wat0 = singles.tile([128, 128], f32)
        wat1 = singles.tile([128, 128], f32)
        watb = singles.tile([1, 128], f32)
        nc.sync.dma_start(out=wat0[:], in_=wat_d[0:128, :])
        nc.sync.dma_start(out=wat1[:], in_=wat_d[128:256, :])
        nc.sync.dma_start(out=watb[:], in_=wat_d[256:257, :])

        # ---- value scratch: natural padded rows (TPR, 256)
        vp = dram.tile([TPR, DM], f32)
        zt = singles.tile([128, DM], f32)
        nc.vector.memset(zt[:], 0.0)
        for l, T in enumerate(LENS):
            nc.sync.dma_start(
                out=vp[:][LSTARTP[l] + T:LSTARTP[l] + T + PAD, :],
                in_=zt[:PAD, :])

        # ---- phase A: value projection into vp
        for tt in range(NVT):
            vt = vpool.tile([128, DM], f32, tag="vt")
            nc.sync.dma_start(out=vt[:], in_=vin_d[tt * 128:(tt + 1) * 128, :])
            ps0 = psum.tile([128, 128], f32, tag="tr")
            ps1 = psum.tile([128, 128], f32, tag="tr")
            nc.tensor.transpose(out=ps0[:], in_=vt[:, 0:128], identity=ident[:])
            nc.tensor.transpose(out=ps1[:], in_=vt[:, 128:256], identity=ident[:])
            vT0 = vpool.tile([128, 128], f32, tag="vT")
            vT1 = vpool.tile([128, 128], f32, tag="vT")
            nc.scalar.copy(out=vT0[:], in_=ps0[:])
            nc.scalar.copy(out=vT1[:], in_=ps1[:])
            pv = psum.tile([128, DM], f32, tag="mm")
            nc.tensor.matmul(out=pv[:], lhsT=vT0[:], rhs=wv0[:], start=True, stop=False)
            nc.tensor.matmul(out=pv[:], lhsT=vT1[:], rhs=wv1[:], start=False, stop=False)
            nc.tensor.matmul(out=pv[:], lhsT=ones_row[:], rhs=wvb[:], start=False, stop=True)
            st = vpool.tile([128, DM], f32, tag="st")
            nc.scalar.copy(out=st[:], in_=pv[:])
            row0 = tt * 128
            acc = 0
            for li, T in enumerate(LENS):
                if row0 < acc + T:
                    l, trel = li, row0 - acc
                    break
                acc += T
            dst = LSTARTP[l] + trel
            nc.sync.dma_start(out=vp[:][dst:dst + 128, :], in_=st[:])

        # ---- phase B: per query tile (optionally unrolled repeats for timing)
        rep = max(1, int(os.environ.get("DEFORM_REPEAT", "1")))
        for qt in [i % NQT for i in range(rep * NQT)]:
            qtile = qpool.tile([128, DM], f32, tag="qtile")
            reft = qpool.tile([128, L], f32, tag="reft")
            nc.sync.dma_start(out=qtile[:], in_=q_d[qt * 128:(qt + 1) * 128, :])
            nc.sync.dma_start(out=reft[:], in_=ref_d[qt * 128:(qt + 1) * 128, :])

            psq0 = psum.tile([128, 128], f32, tag="tr")
            psq1 = psum.tile([128, 128], f32, tag="tr")
            nc.tensor.transpose(out=psq0[:], in_=qtile[:, 0:128], identity=ident[:])
            nc.tensor.transpose(out=psq1[:], in_=qtile[:, 128:256], identity=ident[:])
            qT0 = qpool.tile([128, 128], f32, tag="qT")
            qT1 = qpool.tile([128, 128], f32, tag="qT")
            nc.scalar.copy(out=qT0[:], in_=psq0[:])
            nc.scalar.copy(out=qT1[:], in_=psq1[:])

            offp = psum.tile([128, 128], f32, tag="mm")
            nc.tensor.matmul(out=offp[:], lhsT=qT0[:], rhs=wof0[:], start=True, stop=False)
            nc.tensor.matmul(out=offp[:], lhsT=qT1[:], rhs=wof1[:], start=False, stop=False)
            nc.tensor.matmul(out=offp[:], lhsT=ones_row[:], rhs=wofb[:], start=False, stop=True)
            attp = psum.tile([128, 128], f32, tag="mm")
            nc.tensor.matmul(out=attp[:], lhsT=qT0[:], rhs=wat0[:], start=True, stop=False)
            nc.tensor.matmul(out=attp[:], lhsT=qT1[:], rhs=wat1[:], start=False, stop=False)
            nc.tensor.matmul(out=attp[:], lhsT=ones_row[:], rhs=watb[:], start=False, stop=True)

            # softmax (no max-sub: |logits| < ~4)
            E = qpool.tile([128, 128], f32, tag="E")
            nc.scalar.activation(out=E[:], in_=attp[:], func=ACT.Exp)
            sm = qpool.tile([128, M], f32, tag="sm")
            nc.vector.tensor_reduce(out=sm[:], in_=E[:].rearrange("p (m k) -> p m k", m=M),
                                    axis=mybir.AxisListType.X, op=ALU.add)
            rr = qpool.tile([128, M], f32, tag="rr")
            nc.vector.reciprocal(out=rr[:], in_=sm[:])
            A = qpool.tile([128, 128], f32, tag="A")
            nc.vector.tensor_tensor(out=A[:], in0=E[:],
                                    in1=_ap(rr[:], [[1, M], [0, 16]]), op=ALU.mult)

            # ix = (ref + offs/T)*T - 0.5
            X = qpool.tile([128, 128], f32, tag="X")
            nc.vector.tensor_tensor(out=X[:], in0=offp[:],
                                    in1=_ap(reft[:], [[0, M], [1, L], [0, P]]),
                                    op=ALU.add)
            IX = qpool.tile([128, 128], f32, tag="IX")
            nc.vector.tensor_tensor(out=IX[:], in0=X[:],
                                    in1=consts[:, C_TVEC:C_TVEC + 128], op=ALU.mult)
            nc.vector.tensor_scalar(out=IX[:], in0=IX[:], scalar1=0.5, scalar2=None,
                                    op0=ALU.subtract)

            # needlo per point: relu -> floor (int cast) ; dead mask via relu
            REL = qpool.tile([128, 128], f32, tag="REL")
            nc.vector.tensor_scalar(out=REL[:], in0=IX[:], scalar1=0.0, scalar2=None,
                                    op0=ALU.max)
            FLI = qpool.tile([128, 128], i32, tag="FLI")
            nc.vector.tensor_copy(out=FLI[:], in_=REL[:])
            FLR = qpool.tile([128, 128], f32, tag="FLR")
            nc.vector.tensor_copy(out=FLR[:], in_=FLI[:])
            GT = qpool.tile([128, 128], f32, tag="GT")
            nc.vector.tensor_tensor(out=GT[:], in0=FLR[:], in1=REL[:], op=ALU.is_gt)
            FL = qpool.tile([128, 128], f32, tag="FL")
            nc.vector.tensor_tensor(out=FL[:], in0=FLR[:], in1=GT[:], op=ALU.subtract)
            MSK = qpool.tile([128, 128], f32, tag="MSK")
            nc.scalar.activation(out=MSK[:], in_=IX[:], func=ACT.Relu,
                                 bias=consts[:, C_NEG1:C_NEG1 + 1], scale=-1.0)
            nc.vector.tensor_scalar(out=MSK[:], in0=MSK[:], scalar1=1e13,
                                    scalar2=BIG, op0=ALU.mult, op1=ALU.min)
            NL = qpool.tile([128, 128], f32, tag="NL")
            nc.vector.tensor_tensor(out=NL[:], in0=MSK[:], in1=FL[:], op=ALU.add)
            BMIN = qpool.tile([128, 32], f32, tag="BMIN")
            nc.vector.tensor_reduce(out=BMIN[:],
                                    in_=NL[:].rearrange("p (c k) -> p c k", k=P),
                                    axis=mybir.AxisListType.X, op=ALU.min)
            # min over heads -> (128, L); clamp to T-1
            BM2 = qpool.tile([128, L], f32, tag="BM2")
            nc.vector.tensor_reduce(out=BM2[:],
                                    in_=_ap(BMIN[:], [[1, L], [4, M]]),
                                    axis=mybir.AxisListType.X, op=ALU.min)
            BASEL = qpool.tile([128, L], f32, tag="BASEL")
            nc.vector.tensor_tensor(out=BASEL[:], in0=BM2[:],
                                    in1=consts[:, C_TM1L:C_TM1L + L], op=ALU.min)

            # gather row indices
            IDXF = qpool.tile([128, L], f32, tag="IDXF")
            nc.vector.tensor_tensor(out=IDXF[:], in0=BASEL[:],
                                    in1=consts[:, C_LST:C_LST + L], op=ALU.add)
            IDX = qpool.tile([128, L], i32, tag="IDX")
            nc.vector.tensor_copy(out=IDX[:], in_=IDXF[:])

            # z = ix - base (all-head base per (q,l))
            Z = qpool.tile([128, 128], f32, tag="Z")
            nc.vector.tensor_tensor(out=Z[:], in0=IX[:],
                                    in1=_ap(BASEL[:], [[0, M], [1, L], [0, P]]),
                                    op=ALU.subtract)

            LSTG = spool.tile([128, 1024], f32, tag="LSTG")
            # levels in groups (0,2), (1,3): per-level ops (ISA: <=3 free dims),
            # one 5D pool-avg per group reduces j for both levels at once
            for grp in ((0, 2), (1, 3)):
                W = WCONF[grp[0]]
                LS = grp[1] - grp[0]
                PRW = M * 16 * DH
                PR = spool.tile([128, 2 * M * 16 * DH], f32, tag="PR")
                G = gpool.tile([128, 2 * W * DM], f32, tag=f"G{grp[0]}")
                for gi, l in enumerate(grp):
                    nf = M * P * W
                    D = spool.tile([128, M * P * 10], f32, tag="D")
                    nc.vector.tensor_tensor(
                        out=D[:, :nf],
                        in0=_ap(Z[:], [[16, M], [1, P], [0, W]], extra_offset=l * P),
                        in1=_ap(consts[:], [[0, M], [0, P], [1, W]],
                                extra_offset=C_JROW),
                        op=ALU.subtract)
                    AB = spool.tile([128, M * P * 10], f32, tag="AB")
                    nc.scalar.activation(out=AB[:, :nf], in_=D[:, :nf], func=ACT.Abs)
                    H = spool.tile([128, M * P * 10], f32, tag="H")
                    nc.scalar.activation(out=H[:, :nf], in_=AB[:, :nf], func=ACT.Relu,
                                         bias=1.0, scale=-1.0)
                    HA = spool.tile([128, M * P * 10], f32, tag="HA")
                    nc.vector.tensor_tensor(
                        out=HA[:, :nf], in0=H[:, :nf],
                        in1=_ap(A[:], [[16, M], [1, P], [0, W]], extra_offset=l * P),
                        op=ALU.mult)
                    U2 = spool.tile([128, M * 2 * 10], f32, tag="U2")
                    nc.vector.tensor_tensor(
                        out=U2[:, :M * 2 * W],
                        in0=_ap(HA[:], [[P * W, M], [W, 2], [1, W]]),
                        in1=_ap(HA[:], [[P * W, M], [W, 2], [1, W]],
                                extra_offset=2 * W),
                        op=ALU.add)
                    U = spool.tile([128, M * 10], f32, tag="U")
                    nc.vector.tensor_tensor(
                        out=U[:, :M * W],
                        in0=_ap(U2[:], [[2 * W, M], [1, W]]),
                        in1=_ap(U2[:], [[2 * W, M], [1, W]], extra_offset=W),
                        op=ALU.add)
                    # gather W full rows per query
                    if os.environ.get("DEFORM_NO_GATHER"):
                        nc.vector.memset(G[:, gi * W * DM:(gi + 1) * W * DM], 0.0)
                    else:
                        nc.gpsimd.indirect_dma_start(
                            out=G[:, gi * W * DM:(gi + 1) * W * DM],
                            out_offset=None,
                            in_=vp[:],
                            in_offset=bass.IndirectOffsetOnAxis(
                                ap=IDX[:, l:l + 1], axis=0),
                            bounds_check=TPR - 1,
                            oob_is_err=False,
                        )
                    # PROD[q, m, j, d] = G[q, j, m, d] * U[q, m, j]
                    muleng = nc.vector if (grp[0] == 0 or os.environ.get('DEFORM_ALL_DVE')) else nc.gpsimd
                    muleng.tensor_tensor(
                        out=_ap(PR[:], [[16 * DH, M], [DH, W], [1, DH]],
                                extra_offset=gi * PRW),
                        in0=_ap(G[:], [[DH, M], [DM, W], [1, DH]],
                                extra_offset=gi * W * DM),
                        in1=_ap(U[:], [[W, M], [1, W], [0, DH]]),
                        op=ALU.mult)
                    # j-tree sum on the other engine; final stage -> LSTG col l
                    eng = nc.vector if os.environ.get('DEFORM_ALL_DVE') else (nc.gpsimd if grp[0] == 0 else nc.vector)
                    w = W
                    while w > 1:
                        h = w // 2
                        last = (h == 1) and (w % 2 == 0)
                        po = gi * PRW
                        dst = (_ap(LSTG[:], [[4, M * DH]], extra_offset=l)
                               if last else
                               _ap(PR[:], [[16 * DH, M], [DH, h], [1, DH]],
                                   extra_offset=po))
                        eng.tensor_tensor(
                            out=dst,
                            in0=_ap(PR[:], [[16 * DH, M], [DH, h], [1, DH]],
                                    extra_offset=po),
                            in1=_ap(PR[:], [[16 * DH, M], [DH, h], [1, DH]],
                                    extra_offset=po + h * DH),
                            op=ALU.add)
                        if w % 2:
                            last2 = h == 1
                            dst2 = (_ap(LSTG[:], [[4, M * DH]], extra_offset=l)
                                    if last2 else
                                    _ap(PR[:], [[16 * DH, M], [1, DH]],
                                        extra_offset=po))
                            eng.tensor_tensor(
                                out=dst2,
                                in0=_ap(PR[:], [[16 * DH, M], [1, DH]],
                                        extra_offset=po),
                                in1=_ap(PR[:], [[16 * DH, M], [1, DH]],
                                        extra_offset=po + (w - 1) * DH),
                                op=ALU.add)
                        w = h

            # sum over levels: LSTG (128, (m,d), 4) -> OUTT (128, 256)
            lse = nc.vector if os.environ.get('DEFORM_ALL_DVE') else nc.gpsimd
            T0 = spool.tile([128, DM], f32, tag="T0")
            lse.tensor_tensor(out=T0[:],
                                    in0=_ap(LSTG[:], [[4, M * DH]]),
                                    in1=_ap(LSTG[:], [[4, M * DH]], extra_offset=1),
                                    op=ALU.add)
            T1 = spool.tile([128, DM], f32, tag="T1")
            lse.tensor_tensor(out=T1[:],
                                    in0=_ap(LSTG[:], [[4, M * DH]], extra_offset=2),
                                    in1=_ap(LSTG[:], [[4, M * DH]], extra_offset=3),
                                    op=ALU.add)
            OUTT = spool.tile([128, DM], f32, tag="OUTT")
            lse.tensor_tensor(out=OUTT[:], in0=T0[:], in1=T1[:], op=ALU.add)
            nc.sync.dma_start(out=out_d[qt * 128:(qt + 1) * 128, :], in_=OUTT[:])

    nc.compile()
    return nc


def host_prep(inputs):
    """Build per-core in_maps from full inputs."""
    q = np.ascontiguousarray(inputs["query"], np.float32)
    ref = np.ascontiguousarray(np.asarray(inputs["reference_points"])[..., 0], np.float32)
    vin = np.ascontiguousarray(inputs["input_flatten"], np.float32)
    W_val = np.asarray(inputs["W_val"], np.float32)
    b_val = np.asarray(inputs["b_val"], np.float32)
    W_off = np.asarray(inputs["W_off"], np.float32)
    b_off = np.asarray(inputs["b_off"], np.float32)
    W_attn = np.asarray(inputs["W_attn"], np.float32)
    b_attn = np.asarray(inputs["b_attn"], np.float32)

    Tvec = np.zeros(M * L * P, np.float32)
    for c in range(M * L * P):
        Tvec[c] = LENS[(c % 16) // 4]
    wv = np.concatenate([W_val.T, b_val[None, :]], 0)
    wof = np.concatenate([(W_off / Tvec[:, None]).T, (b_off / Tvec)[None, :]], 0)
    wat = np.concatenate([W_attn.T, b_attn[None, :]], 0)

    consts = np.zeros((1, CW), np.float32)
    consts[0, C_TVEC:C_TVEC + 128] = Tvec
    for l in range(L):
        consts[0, C_TM1L + l] = LENS[l] - 1
        consts[0, C_LST + l] = LSTARTP[l]
    consts[0, C_JROW:C_JROW + 16] = np.arange(16, dtype=np.float32)
    consts[0, C_NEG1] = -1.0

    shared = {"wv": np.ascontiguousarray(wv), "wof": np.ascontiguousarray(wof),
              "wat": np.ascontiguousarray(wat), "consts": consts}
    return [
        {"q": q[n], "ref": ref[n], "vin": vin[n], **shared}
        for n in range(N)
    ]


_NC_CACHE = None


def kernel(**inputs) -> np.ndarray:
    global _NC_CACHE
    if _NC_CACHE is None:
        _NC_CACHE = build_program()
    nc = _NC_CACHE
    in_maps = host_prep(inputs)
    res = run_bass_kernel_spmd(nc, in_maps, list(range(N)))
    return np.stack([res.results[n]["out"] for n in range(N)]).astype(np.float32)


if __name__ == "__main__":
    d = np.load("/root/problem/cached_io.npz")
    inp = {k: d[k] for k in ["query", "reference_points", "input_flatten",
                             "input_temporal_lens", "input_level_start_index",
                             "W_val", "b_val", "W_off", "b_off", "W_attn", "b_attn"]}
    out = kernel(**inp)
    ref = d["ref_out"]
    err = np.abs(out - ref).max()
    print("absmax err:", err, "scale:", np.abs(ref).max(),
          "rel:", err / np.abs(ref).max())



# revision 9
# speedup vs baseline: 1.1214x; 1.1214x over previous
"""Trainium2 Bass kernel for 1D multi-scale deformable attention.

Self-contained: builds the Bass/Tile program, shards the full inputs
data-parallel over N across 8 NeuronCores, runs via run_bass_kernel_spmd,
and returns the full (N, LQ, 256) output.

Algorithm per core (one batch element):
  value = vin @ W_val.T + b_val        -> bf16 padded rows (T', 256) in DRAM
  ix    = q @ W_off.T + (b_off - 0.5) + ref*T   (ref*T via PE selector matmul)
  attn  = softmax(q @ W_attn.T + b_attn) per (q, m) over 16 (l,p)
  bilinear + zero padding == sum_t relu(1 - |ix - t|) * V[t]
  per q: base_l = clamp(floor(relu(min_{m,p} ix)), T-1); ONE indirect DMA
  gathers 4 levels x 10 rows x 512B bf16 per query
  U[m,l,j] = sum_p attn * tri(z - j)   (pool over p, /4 refolded later)
  PROD[l,m,d,j] = 4 * G * U ; out = sum_{l,j} PROD
"""
import os
import numpy as np
from contextlib import ExitStack

import concourse.bass as bass
import concourse.bacc as bacc
import concourse.tile as tile
from concourse import mybir
from concourse.bass_utils import run_bass_kernel_spmd

f32 = mybir.dt.float32
bf16 = mybir.dt.bfloat16
i32 = mybir.dt.int32
ALU = mybir.AluOpType
ACT = mybir.ActivationFunctionType

# static problem config
LENS = (2048, 1024, 512, 256)
N, LQ, DM = 8, 2048, 256
M, L, P, DH = 8, 4, 4, 32
S = sum(LENS)                      # 3840
WG = 10                            # uniform window rows per level
PAD = 12                           # zero rows after each level (>= WG-1)
LSTARTP = []
_s = 0
for _T in LENS:
    LSTARTP.append(_s)
    _s += _T + PAD
TPR = _s                           # 3888 padded rows total
NQT = LQ // 128                    # 16 query tiles
NVT = S // 128                     # 30 value tiles

# consts layout (one row, broadcast to 128 partitions at load)
C_JROW = 0           # 16: j = 0..15
C_TM1L = 16          # 4:  T_l - 1
C_LST = 20           # 4:  LSTARTP[l]
CW = 24


def _ap(base, dims, extra_offset=0):
    """Custom strided AP derived from a 2D (128, F) contiguous tile AP."""
    return bass.AP(
        tensor=base.tensor,
        offset=base.offset + extra_offset,
        ap=[list(base.ap[0])] + [[s, c] for s, c in dims],
    )


def build_program():
    nc = bacc.Bacc("TRN2", target_bir_lowering=False, debug=False)

    qT_d = nc.dram_tensor("qT", [DM, LQ], f32, kind="ExternalInput")
    refT_d = nc.dram_tensor("refT", [L, LQ], f32, kind="ExternalInput")
    vinT_d = nc.dram_tensor("vinT", [DM, S], f32, kind="ExternalInput")
    wv_d = nc.dram_tensor("wv", [DM + 1, DM], f32, kind="ExternalInput")
    wof_d = nc.dram_tensor("wof", [DM, M * L * P], f32, kind="ExternalInput")
    wofsel_d = nc.dram_tensor("wofsel", [1 + L, M * L * P], f32,
                              kind="ExternalInput")
    wat_d = nc.dram_tensor("wat", [DM + 1, M * L * P], f32, kind="ExternalInput")
    consts_d = nc.dram_tensor("consts", [1, CW], f32, kind="ExternalInput")
    out_d = nc.dram_tensor("out", [LQ, DM], f32, kind="ExternalOutput")

    with tile.TileContext(nc) as tc, ExitStack() as ctx:
        singles = ctx.enter_context(tc.tile_pool(name="singles", bufs=1))
        dram = ctx.enter_context(tc.tile_pool(name="dram", bufs=1, space="DRAM"))
        vpool = ctx.enter_context(tc.tile_pool(name="vpool", bufs=3))
        psum = ctx.enter_context(tc.tile_pool(name="psum", bufs=2, space="PSUM"))
        qpool = ctx.enter_context(tc.tile_pool(name="qpool", bufs=2))
        gpool = ctx.enter_context(tc.tile_pool(name="gpool", bufs=2))
        spool = ctx.enter_context(tc.tile_pool(name="spool", bufs=2))

        # ---- constants / weights (loaded once)
        consts = singles.tile([128, CW], f32)
        nc.sync.dma_start(
            out=consts[:],
            in_=bass.AP(tensor=consts_d[:].tensor, offset=0,
                        ap=[[0, 128], [1, CW]]),
        )
        wv0 = singles.tile([128, DM], f32)
        wv1 = singles.tile([128, DM], f32)
        wvb = singles.tile([1, DM], f32)
        nc.sync.dma_start(out=wv0[:], in_=wv_d[0:128, :])
        nc.sync.dma_start(out=wv1[:], in_=wv_d[128:256, :])
        nc.sync.dma_start(out=wvb[:], in_=wv_d[256:257, :])
        wof0 = singles.tile([128, 128], f32)
        wof1 = singles.tile([128, 128], f32)
        wofsel = singles.tile([1 + L, 128], f32)
        nc.sync.dma_start(out=wof0[:], in_=wof_d[0:128, :])
        nc.sync.dma_start(out=wof1[:], in_=wof_d[128:256, :])
        nc.sync.dma_start(out=wofsel[:], in_=wofsel_d[:, :])
        wat0 = singles.tile([128, 128], f32)
        wat1 = singles.tile([128, 128], f32)
        watb = singles.tile([1, 128], f32)
        nc.sync.dma_start(out=wat0[:], in_=wat_d[0:128, :])
        nc.sync.dma_start(out=wat1[:], in_=wat_d[128:256, :])
        nc.sync.dma_start(out=watb[:], in_=wat_d[256:257, :])

        # augmented lhsT rows: [ones; refT] (5, LQ)
        aug = singles.tile([1 + L, LQ], f32)
        nc.vector.memset(aug[:][0:1, :], 1.0)
        nc.sync.dma_start(out=aug[:][1:1 + L, :], in_=refT_d[:, :])

        # full q^T and vin^T resident in SBUF
        qT0 = singles.tile([128, LQ], f32)
        qT1 = singles.tile([128, LQ], f32)
        nc.sync.dma_start(out=qT0[:], in_=qT_d[0:128, :])
        nc.sync.dma_start(out=qT1[:], in_=qT_d[128:256, :])
        vinT0 = singles.tile([128, S], f32)
        vinT1 = singles.tile([128, S], f32)
        nc.sync.dma_start(out=vinT0[:], in_=vinT_d[0:128, :])
        nc.sync.dma_start(out=vinT1[:], in_=vinT_d[128:256, :])

        # ---- value scratch: natural padded rows (TPR, 256) bf16
        vp = dram.tile([TPR, DM], bf16)
        zt = singles.tile([128, DM], bf16)
        nc.vector.memset(zt[:], 0.0)
        for l, T in enumerate(LENS):
            nc.sync.dma_start(
                out=vp[:][LSTARTP[l] + T:LSTARTP[l] + T + PAD, :],
                in_=zt[:PAD, :])

        # ---- phase A: value projection into vp (bf16)
        for tt in range(NVT):
            pv = psum.tile([128, DM], f32, tag="pv")
            nc.tensor.matmul(out=pv[:], lhsT=vinT0[:, tt * 128:(tt + 1) * 128],
                             rhs=wv0[:], start=True, stop=False)
            nc.tensor.matmul(out=pv[:], lhsT=vinT1[:, tt * 128:(tt + 1) * 128],
                             rhs=wv1[:], start=False, stop=False)
            nc.tensor.matmul(out=pv[:], lhsT=aug[0:1, 0:128], rhs=wvb[:],
                             start=False, stop=True)
            st = vpool.tile([128, DM], bf16, tag="st")
            nc.scalar.copy(out=st[:], in_=pv[:])
            row0 = tt * 128
            acc = 0
            for li, T in enumerate(LENS):
                if row0 < acc + T:
                    l, trel = li, row0 - acc
                    break
                acc += T
            dst = LSTARTP[l] + trel
            nc.sync.dma_start(out=vp[:][dst:dst + 128, :], in_=st[:])

        # ---- phase B: per query tile
        for qt in range(NQT):
            qs = slice(qt * 128, (qt + 1) * 128)

            offp = psum.tile([128, 128], f32, tag="off")
            nc.tensor.matmul(out=offp[:], lhsT=qT0[:, qs], rhs=wof0[:],
                             start=True, stop=False)
            nc.tensor.matmul(out=offp[:], lhsT=qT1[:, qs], rhs=wof1[:],
                             start=False, stop=False)
            nc.tensor.matmul(out=offp[:], lhsT=aug[:, qs], rhs=wofsel[:],
                             start=False, stop=True)
            attp = psum.tile([128, 128], f32, tag="att")
            nc.tensor.matmul(out=attp[:], lhsT=qT0[:, qs], rhs=wat0[:],
                             start=True, stop=False)
            nc.tensor.matmul(out=attp[:], lhsT=qT1[:, qs], rhs=wat1[:],
                             start=False, stop=False)
            nc.tensor.matmul(out=attp[:], lhsT=aug[0:1, qs], rhs=watb[:],
                             start=False, stop=True)

            # softmax (no max-sub: |logits| < ~4) -> A (bf16)
            E = qpool.tile([128, 128], f32, tag="E")
            nc.scalar.activation(out=E[:], in_=attp[:], func=ACT.Exp)
            sm = qpool.tile([128, M], f32, tag="sm")
            nc.vector.tensor_reduce(out=sm[:], in_=_ap(E[:], [[16, M], [1, 16]]),
                                    axis=mybir.AxisListType.X, op=ALU.add)
            rr = qpool.tile([128, M], f32, tag="rr")
            nc.vector.reciprocal(out=rr[:], in_=sm[:])
            A = qpool.tile([128, 128], bf16, tag="A")
            nc.vector.tensor_tensor(out=A[:], in0=E[:],
                                    in1=_ap(rr[:], [[1, M], [0, 16]]), op=ALU.mult)

            # base_l = clamp(floor(relu(min_{m,p} ix)), T-1); ix == offp
            BM2 = qpool.tile([128, L], f32, tag="BM2")
            nc.vector.tensor_reduce(out=BM2[:],
                                    in_=_ap(offp[:], [[4, L], [16, M], [1, P]]),
                                    axis=mybir.AxisListType.XY, op=ALU.min)
            REL = qpool.tile([128, L], f32, tag="REL")
            nc.scalar.activation(out=REL[:], in_=BM2[:], func=ACT.Relu)
            FLI = qpool.tile([128, L], i32, tag="FLI")
            nc.vector.tensor_copy(out=FLI[:], in_=REL[:])
            FLR = qpool.tile([128, L], f32, tag="FLR")
            nc.vector.tensor_copy(out=FLR[:], in_=FLI[:])
            GT = qpool.tile([128, L], f32, tag="GT")
            nc.vector.tensor_tensor(out=GT[:], in0=FLR[:], in1=REL[:], op=ALU.is_gt)
            FL = qpool.tile([128, L], f32, tag="FL")
            nc.vector.tensor_tensor(out=FL[:], in0=FLR[:], in1=GT[:], op=ALU.subtract)
            BASEL = qpool.tile([128, L], f32, tag="BASEL")
            nc.vector.tensor_tensor(out=BASEL[:], in0=FL[:],
                                    in1=consts[:, C_TM1L:C_TM1L + L], op=ALU.min)
            IDXF = qpool.tile([128, L], f32, tag="IDXF")
            nc.vector.tensor_tensor(out=IDXF[:], in0=BASEL[:],
                                    in1=consts[:, C_LST:C_LST + L], op=ALU.add)
            IDX = qpool.tile([128, L], i32, tag="IDX")
            nc.vector.tensor_copy(out=IDX[:], in_=IDXF[:])

            # gathers: per level WG rows x 512B bf16 per query
            # (issued early on the gpsimd queue so DMA overlaps tri-eval)
            G = gpool.tile([128, L * WG * DM], bf16, tag="G")
            for l in range(L):
                nc.gpsimd.indirect_dma_start(
                    out=G[:, l * WG * DM:(l + 1) * WG * DM],
                    out_offset=None,
                    in_=vp[:],
                    in_offset=bass.IndirectOffsetOnAxis(ap=IDX[:, l:l + 1],
                                                        axis=0),
                    bounds_check=TPR - 1,
                    oob_is_err=False,
                )

            # z = ix - base  (128, 128) f32
            Z = qpool.tile([128, 128], f32, tag="Z")
            nc.vector.tensor_tensor(out=Z[:], in0=offp[:],
                                    in1=_ap(BASEL[:], [[0, M], [1, L], [0, P]]),
                                    op=ALU.subtract)

            # tri weights over (ml, j, p): D = z - j ; H = relu(1-|D|) bf16
            D = spool.tile([128, M * L * WG * P], f32, tag="D")
            nc.gpsimd.tensor_tensor(
                out=D[:],
                in0=_ap(Z[:], [[4, M * L], [0, WG], [1, P]]),
                in1=_ap(consts[:], [[0, M * L], [1, WG], [0, P]],
                        extra_offset=C_JROW),
                op=ALU.subtract)
            AB = spool.tile([128, M * L * WG * P], f32, tag="AB")
            nc.scalar.activation(out=AB[:], in_=D[:], func=ACT.Abs)
            H = spool.tile([128, M * L * WG * P], bf16, tag="H")
            nc.scalar.activation(out=H[:], in_=AB[:], func=ACT.Relu,
                                 bias=1.0, scale=-1.0)
            HA = spool.tile([128, M * L * WG * P], bf16, tag="HA")
            nc.vector.tensor_tensor(
                out=HA[:], in0=H[:],
                in1=_ap(A[:], [[4, M * L], [0, WG], [1, P]]),
                op=ALU.mult)
            # U[(ml), j] = sum_p HA
            UF = spool.tile([128, M * L * WG], f32, tag="UF")
            nc.vector.tensor_reduce(
                out=UF[:],
                in_=_ap(HA[:], [[WG * P, M * L], [P, WG], [1, P]]),
                axis=mybir.AxisListType.X, op=ALU.add)
            U = spool.tile([128, M * L * WG], bf16, tag="U")
            nc.scalar.copy(out=U[:], in_=UF[:])

            # PROD[l, m, d, j] = 4 * G * U ; sum over (j) then (l)
            PR = spool.tile([128, L * M * DH * WG], bf16, tag="PR")
            for l in range(L):
                eng = nc.vector if l in (0, 2) else nc.gpsimd
                eng.tensor_tensor(
                    out=_ap(PR[:], [[DH * WG, M], [WG, DH], [1, WG]],
                            extra_offset=l * M * DH * WG),
                    in0=_ap(G[:], [[DH, M], [1, DH], [DM, WG]],
                            extra_offset=l * WG * DM),
                    in1=_ap(U[:], [[L * WG, M], [0, DH], [1, WG]],
                            extra_offset=l * WG),
                    op=ALU.mult)
            LSTG = spool.tile([128, L * DM], f32, tag="LSTG")
            nc.vector.tensor_reduce(
                out=LSTG[:],
                in_=_ap(PR[:], [[WG, L * M * DH], [1, WG]]),
                axis=mybir.AxisListType.X, op=ALU.add)
            OUTT = spool.tile([128, DM], f32, tag="OUTT")
            nc.vector.tensor_reduce(
                out=OUTT[:],
                in_=_ap(LSTG[:], [[1, DM], [DM, L]]),
                axis=mybir.AxisListType.X, op=ALU.add)
            nc.sync.dma_start(out=out_d[qs, :], in_=OUTT[:])

    nc.compile()
    return nc


def host_prep(inputs):
    """Build per-core in_maps from full inputs."""
    q = np.ascontiguousarray(inputs["query"], np.float32)
    ref = np.asarray(inputs["reference_points"], np.float32)[..., 0]  # (N,LQ,L)
    vin = np.ascontiguousarray(inputs["input_flatten"], np.float32)
    W_val = np.asarray(inputs["W_val"], np.float32)
    b_val = np.asarray(inputs["b_val"], np.float32)
    W_off = np.asarray(inputs["W_off"], np.float32)
    b_off = np.asarray(inputs["b_off"], np.float32)
    W_attn = np.asarray(inputs["W_attn"], np.float32)
    b_attn = np.asarray(inputs["b_attn"], np.float32)

    wv = np.concatenate([W_val.T, b_val[None, :]], 0)
    wof = np.ascontiguousarray(W_off.T)
    # row 0: b_off - 0.5 ; rows 1..4: SEL[l, c] = T_l * [level(c) == l]
    wofsel = np.zeros((1 + L, M * L * P), np.float32)
    wofsel[0] = b_off - 0.5
    for c in range(M * L * P):
        l = (c % 16) // 4
        wofsel[1 + l, c] = LENS[l]
    wat = np.concatenate([W_attn.T, b_attn[None, :]], 0)

    consts = np.zeros((1, CW), np.float32)
    consts[0, C_JROW:C_JROW + 16] = np.arange(16, dtype=np.float32)
    for l in range(L):
        consts[0, C_TM1L + l] = LENS[l] - 1
        consts[0, C_LST + l] = LSTARTP[l]

    shared = {"wv": np.ascontiguousarray(wv), "wof": wof,
              "wofsel": wofsel, "wat": np.ascontiguousarray(wat),
              "consts": consts}
    return [
        {"qT": np.ascontiguousarray(q[n].T),
         "refT": np.ascontiguousarray(ref[n].T),
         "vinT": np.ascontiguousarray(vin[n].T), **shared}
        for n in range(N)
    ]


_NC_CACHE = None


def kernel(**inputs) -> np.ndarray:
    global _NC_CACHE
    if _NC_CACHE is None:
        _NC_CACHE = build_program()
    nc = _NC_CACHE
    in_maps = host_prep(inputs)
    res = run_bass_kernel_spmd(nc, in_maps, list(range(N)))
    return np.stack([res.results[n]["out"] for n in range(N)]).astype(np.float32)


if __name__ == "__main__":
    d = np.load("/root/problem/cached_io.npz")
    inp = {k: d[k] for k in ["query", "reference_points", "input_flatten",
                             "input_temporal_lens", "input_level_start_index",
                             "W_val", "b_val", "W_off", "b_off", "W_attn", "b_attn"]}
    out = kernel(**inp)
    ref = d["ref_out"]
    err = np.abs(out - ref).max()
    print("absmax err:", err, "scale:", np.abs(ref).max(),
          "rel:", err / np.abs(ref).max())


# revision 10
# speedup vs baseline: 1.8918x; 1.6871x over previous
"""Trainium2 Bass kernel for 1D multi-scale deformable attention.

Self-contained: builds the Bass/Tile program, shards the full inputs
data-parallel over N across 8 NeuronCores, runs via run_bass_kernel_spmd,
and returns the full (N, LQ, 256) output.

Algorithm per core (one batch element):
  value = vin @ W_val.T + b_val        -> bf16 padded rows (T', 256) in DRAM
  ix    = q @ W_off.T + (b_off - 0.5) + ref*T   (ref*T via PE selector matmul)
  attn  = softmax(q @ W_attn.T + b_attn) per (q, m) over 16 (l,p)
  bilinear + zero padding == sum_t relu(1 - |ix - t|) * V[t]
  per q: base_l = clamp(floor(relu(min_{m,p} ix)), T-1); per-level indirect
  DMA gathers WG rows x 512B bf16 per query into G (l, j, m*d) bf16
  tri pipeline in (j, c) layout (c = m*16+l*4+p) for long contiguous runs:
    D = Z - j ; H = relu(1-|D|) ; HA = H*A ; U = sum_p HA
  U reordered (l,j,m) and expanded over d on ScalarE -> U32 (l,j,m,d)
  PROD = G * U32 (packed bf16 2x) ; out = tree-sum over l then j
"""
import os
import numpy as np
from contextlib import ExitStack

import concourse.bass as bass
import concourse.bacc as bacc
import concourse.tile as tile
from concourse import mybir
from concourse.bass_utils import run_bass_kernel_spmd

f32 = mybir.dt.float32
bf16 = mybir.dt.bfloat16
i32 = mybir.dt.int32
ALU = mybir.AluOpType
ACT = mybir.ActivationFunctionType

# static problem config
LENS = (2048, 1024, 512, 256)
N, LQ, DM = 8, 2048, 256
M, L, P, DH = 8, 4, 4, 32
S = sum(LENS)                      # 3840
WG = 10                            # uniform window rows per level
PAD = 12                           # zero rows after each level (>= WG-1)
LSTARTP = []
_s = 0
for _T in LENS:
    LSTARTP.append(_s)
    _s += _T + PAD
TPR = _s                           # 3888 padded rows total
NQT = LQ // 128                    # 16 query tiles
NVT = S // 128                     # 30 value tiles
GW = WG * DM                       # gathered elems per (q, l)

# consts layout (one row, broadcast to 128 partitions at load)
C_TM1L = 0           # 4:  T_l - 1
C_LST = 4            # 4:  LSTARTP[l]
CW = 8


def _ap(base, dims, extra_offset=0):
    """Custom strided AP derived from a 2D (128, F) contiguous tile AP."""
    return bass.AP(
        tensor=base.tensor,
        offset=base.offset + extra_offset,
        ap=[list(base.ap[0])] + [[s, c] for s, c in dims],
    )


def build_program():
    nc = bacc.Bacc("TRN2", target_bir_lowering=False, debug=False)

    qT_d = nc.dram_tensor("qT", [DM, LQ], f32, kind="ExternalInput")
    refT_d = nc.dram_tensor("refT", [L, LQ], f32, kind="ExternalInput")
    vinT_d = nc.dram_tensor("vinT", [DM, S], bf16, kind="ExternalInput")
    wv_d = nc.dram_tensor("wv", [DM, DM], bf16, kind="ExternalInput")
    wvb_d = nc.dram_tensor("wvb", [1, DM], f32, kind="ExternalInput")
    wof_d = nc.dram_tensor("wof", [DM, M * L * P], f32, kind="ExternalInput")
    wofsel_d = nc.dram_tensor("wofsel", [1 + L, M * L * P], f32,
                              kind="ExternalInput")
    wat_d = nc.dram_tensor("wat", [DM + 1, M * L * P], f32, kind="ExternalInput")
    consts_d = nc.dram_tensor("consts", [1, CW], f32, kind="ExternalInput")
    out_d = nc.dram_tensor("out", [LQ, DM], f32, kind="ExternalOutput")

    with tile.TileContext(nc) as tc, ExitStack() as ctx:
        singles = ctx.enter_context(tc.tile_pool(name="singles", bufs=1))
        dram = ctx.enter_context(tc.tile_pool(name="dram", bufs=1, space="DRAM"))
        psum = ctx.enter_context(tc.tile_pool(name="psum", bufs=2, space="PSUM"))
        qpool = ctx.enter_context(tc.tile_pool(name="qpool", bufs=2))
        gpool = ctx.enter_context(tc.tile_pool(name="gpool", bufs=2))
        spool = ctx.enter_context(tc.tile_pool(name="spool", bufs=1))
        dpool = ctx.enter_context(tc.tile_pool(name="dpool", bufs=2))

        # ---- constants / weights (loaded once)
        consts = singles.tile([128, CW], f32)
        nc.sync.dma_start(
            out=consts[:],
            in_=bass.AP(tensor=consts_d[:].tensor, offset=0,
                        ap=[[0, 128], [1, CW]]),
        )
        wv0 = singles.tile([128, DM], bf16)
        wv1 = singles.tile([128, DM], bf16)
        wvb = singles.tile([1, DM], f32)
        nc.sync.dma_start(out=wv0[:], in_=wv_d[0:128, :])
        nc.sync.dma_start(out=wv1[:], in_=wv_d[128:256, :])
        nc.sync.dma_start(out=wvb[:], in_=wvb_d[:, :])
        wof0 = singles.tile([128, 128], f32)
        wof1 = singles.tile([128, 128], f32)
        wofsel = singles.tile([1 + L, 128], f32)
        nc.sync.dma_start(out=wof0[:], in_=wof_d[0:128, :])
        nc.sync.dma_start(out=wof1[:], in_=wof_d[128:256, :])
        nc.sync.dma_start(out=wofsel[:], in_=wofsel_d[:, :])
        wat0 = singles.tile([128, 128], f32)
        wat1 = singles.tile([128, 128], f32)
        watb = singles.tile([1, 128], f32)
        nc.sync.dma_start(out=wat0[:], in_=wat_d[0:128, :])
        nc.sync.dma_start(out=wat1[:], in_=wat_d[128:256, :])
        nc.sync.dma_start(out=watb[:], in_=wat_d[256:257, :])

        # augmented lhsT rows: [ones; refT] (5, LQ)
        aug = singles.tile([1 + L, LQ], f32)
        nc.vector.memset(aug[:][0:1, :], 1.0)
        nc.sync.dma_start(out=aug[:][1:1 + L, :], in_=refT_d[:, :])

        # full q^T resident in SBUF
        qT0 = singles.tile([128, LQ], f32)
        qT1 = singles.tile([128, LQ], f32)
        nc.sync.dma_start(out=qT0[:], in_=qT_d[0:128, :])
        nc.sync.dma_start(out=qT1[:], in_=qT_d[128:256, :])

        # JC[p, j*128 + c] = j  (tri grid in (j, c) layout)
        JC = singles.tile([128, WG * 128], f32)
        for j in range(WG):
            nc.vector.memset(JC[:, j * 128:(j + 1) * 128], float(j))

        # ---- value scratch: natural padded rows (TPR, 256) bf16
        vp = dram.tile([TPR, DM], bf16)
        zt = singles.tile([128, DM], bf16)
        nc.vector.memset(zt[:], 0.0)
        for l, T in enumerate(LENS):
            nc.sync.dma_start(
                out=vp[:][LSTARTP[l] + T:LSTARTP[l] + T + PAD, :],
                in_=zt[:PAD, :])

        # ---- phase A: value projection into vp (bf16, bf16 matmuls)
        with tc.tile_pool(name="vload", bufs=1) as vload, \
             tc.tile_pool(name="vst", bufs=3) as vst:
            vinT0 = vload.tile([128, S], bf16)
            vinT1 = vload.tile([128, S], bf16)
            nc.sync.dma_start(out=vinT0[:], in_=vinT_d[0:128, :])
            nc.sync.dma_start(out=vinT1[:], in_=vinT_d[128:256, :])
            for tt in range(NVT):
                pv = psum.tile([128, DM], f32, tag="pv")
                nc.tensor.matmul(out=pv[:], lhsT=vinT0[:, tt * 128:(tt + 1) * 128],
                                 rhs=wv0[:], start=True, stop=False)
                nc.tensor.matmul(out=pv[:], lhsT=vinT1[:, tt * 128:(tt + 1) * 128],
                                 rhs=wv1[:], start=False, stop=False)
                nc.tensor.matmul(out=pv[:], lhsT=aug[0:1, 0:128], rhs=wvb[:],
                                 start=False, stop=True)
                st = vst.tile([128, DM], bf16, tag="st")
                nc.scalar.copy(out=st[:], in_=pv[:])
                row0 = tt * 128
                acc = 0
                for li, T in enumerate(LENS):
                    if row0 < acc + T:
                        l, trel = li, row0 - acc
                        break
                    acc += T
                dst = LSTARTP[l] + trel
                nc.sync.dma_start(out=vp[:][dst:dst + 128, :], in_=st[:])

        # ---- phase B: per query tile
        for qt in range(NQT):
            qs = slice(qt * 128, (qt + 1) * 128)

            offp = psum.tile([128, 128], f32, tag="off")
            nc.tensor.matmul(out=offp[:], lhsT=qT0[:, qs], rhs=wof0[:],
                             start=True, stop=False)
            nc.tensor.matmul(out=offp[:], lhsT=qT1[:, qs], rhs=wof1[:],
                             start=False, stop=False)
            nc.tensor.matmul(out=offp[:], lhsT=aug[:, qs], rhs=wofsel[:],
                             start=False, stop=True)
            attp = psum.tile([128, 128], f32, tag="att")
            nc.tensor.matmul(out=attp[:], lhsT=qT0[:, qs], rhs=wat0[:],
                             start=True, stop=False)
            nc.tensor.matmul(out=attp[:], lhsT=qT1[:, qs], rhs=wat1[:],
                             start=False, stop=False)
            nc.tensor.matmul(out=attp[:], lhsT=aug[0:1, qs], rhs=watb[:],
                             start=False, stop=True)

            # softmax (no max-sub: |logits| < ~4) -> A (bf16)
            E = qpool.tile([128, 128], f32, tag="E")
            nc.scalar.activation(out=E[:], in_=attp[:], func=ACT.Exp)
            sm = qpool.tile([128, M], f32, tag="sm")
            nc.vector.tensor_reduce(out=sm[:], in_=_ap(E[:], [[16, M], [1, 16]]),
                                    axis=mybir.AxisListType.X, op=ALU.add)
            rr = qpool.tile([128, M], f32, tag="rr")
            nc.vector.reciprocal(out=rr[:], in_=sm[:])
            A = qpool.tile([128, 128], bf16, tag="A")
            nc.vector.tensor_tensor(out=A[:], in0=E[:],
                                    in1=_ap(rr[:], [[1, M], [0, 16]]), op=ALU.mult)

            # base_l = clamp(floor(relu(min_{m,p} ix)), T-1); ix == offp
            BM2 = qpool.tile([128, L], f32, tag="BM2")
            nc.vector.tensor_reduce(out=BM2[:],
                                    in_=_ap(offp[:], [[4, L], [16, M], [1, P]]),
                                    axis=mybir.AxisListType.XY, op=ALU.min)
            REL = qpool.tile([128, L], f32, tag="REL")
            nc.scalar.activation(out=REL[:], in_=BM2[:], func=ACT.Relu)
            FLI = qpool.tile([128, L], i32, tag="FLI")
            nc.vector.tensor_copy(out=FLI[:], in_=REL[:])
            FLR = qpool.tile([128, L], f32, tag="FLR")
            nc.vector.tensor_copy(out=FLR[:], in_=FLI[:])
            GT = qpool.tile([128, L], f32, tag="GT")
            nc.vector.tensor_tensor(out=GT[:], in0=FLR[:], in1=REL[:], op=ALU.is_gt)
            FL = qpool.tile([128, L], f32, tag="FL")
            nc.vector.tensor_tensor(out=FL[:], in0=FLR[:], in1=GT[:], op=ALU.subtract)
            BASEL = qpool.tile([128, L], f32, tag="BASEL")
            nc.vector.tensor_tensor(out=BASEL[:], in0=FL[:],
                                    in1=consts[:, C_TM1L:C_TM1L + L], op=ALU.min)
            IDXF = qpool.tile([128, L], f32, tag="IDXF")
            nc.vector.tensor_tensor(out=IDXF[:], in0=BASEL[:],
                                    in1=consts[:, C_LST:C_LST + L], op=ALU.add)
            IDX = qpool.tile([128, L], i32, tag="IDX")
            nc.vector.tensor_copy(out=IDX[:], in_=IDXF[:])

            # gathers: per level WG rows x 512B bf16 per query -> G (l, j, md)
            G = gpool.tile([128, L * GW], bf16, tag="G")
            for l in range(L):
                nc.gpsimd.indirect_dma_start(
                    out=G[:, l * GW:(l + 1) * GW],
                    out_offset=None,
                    in_=vp[:],
                    in_offset=bass.IndirectOffsetOnAxis(ap=IDX[:, l:l + 1],
                                                        axis=0),
                    bounds_check=TPR - 1,
                    oob_is_err=False,
                )

            # z = ix - base  (128, 128) f32, c-order
            Z = qpool.tile([128, 128], f32, tag="Z")
            nc.vector.tensor_tensor(out=Z[:], in0=offp[:],
                                    in1=_ap(BASEL[:], [[0, M], [1, L], [0, P]]),
                                    op=ALU.subtract)

            # tri weights in (j, c) layout: D = z - j ; H = relu(1-|D|) bf16
            D = dpool.tile([128, WG * 128], f32, tag="D")
            nc.vector.tensor_tensor(
                out=D[:],
                in0=_ap(Z[:], [[0, WG], [1, 128]]),
                in1=JC[:],
                op=ALU.subtract)
            AB = spool.tile([128, WG * 128], f32, tag="AB")
            nc.scalar.activation(out=AB[:], in_=D[:], func=ACT.Abs)
            H = dpool.tile([128, WG * 128], bf16, tag="H")
            nc.scalar.activation(out=H[:], in_=AB[:], func=ACT.Relu,
                                 bias=1.0, scale=-1.0)
            HA = spool.tile([128, WG * 128], bf16, tag="HA")
            nc.vector.tensor_tensor(
                out=HA[:], in0=H[:],
                in1=_ap(A[:], [[0, WG], [1, 128]]),
                op=ALU.mult)
            # UF[j, (m,l)] = sum_p HA ; UT = (l, j, m) bf16 ; U32 = (l, j, m, d)
            UF = dpool.tile([128, WG * M * L], f32, tag="UF")
            nc.vector.tensor_reduce(
                out=UF[:],
                in_=_ap(HA[:], [[128, WG], [4, M * L], [1, P]]),
                axis=mybir.AxisListType.X, op=ALU.add)
            UT = spool.tile([128, L * WG * M], bf16, tag="UT")
            nc.scalar.copy(
                out=UT[:],
                in_=_ap(UF[:], [[1, L], [M * L, WG], [L, M]]))
            U32 = dpool.tile([128, L * WG * M * DH], bf16, tag="U32")
            nc.scalar.copy(
                out=U32[:],
                in_=_ap(UT[:], [[1, L * WG * M], [0, DH]]))

            # PROD (l, j, m, d) = G * U32, packed bf16 2x; tree-sum l then j
            PR = spool.tile([128, L * GW], bf16, tag="PR")
            nc.vector.tensor_tensor(out=PR[:], in0=G[:], in1=U32[:], op=ALU.mult)
            S1 = spool.tile([128, 2 * GW], bf16, tag="S1")
            nc.vector.tensor_tensor(out=S1[:], in0=PR[:, 0:2 * GW],
                                    in1=PR[:, 2 * GW:4 * GW], op=ALU.add)
            S2 = spool.tile([128, GW], bf16, tag="S2")
            nc.vector.tensor_tensor(out=S2[:], in0=S1[:, 0:GW],
                                    in1=S1[:, GW:2 * GW], op=ALU.add)
            T1 = spool.tile([128, 5 * DM], bf16, tag="T1")
            nc.vector.tensor_tensor(out=T1[:], in0=S2[:, 0:5 * DM],
                                    in1=S2[:, 5 * DM:10 * DM], op=ALU.add)
            T2 = spool.tile([128, 2 * DM], bf16, tag="T2")
            nc.vector.tensor_tensor(out=T2[:], in0=T1[:, 0:2 * DM],
                                    in1=T1[:, 2 * DM:4 * DM], op=ALU.add)
            T3 = spool.tile([128, DM], bf16, tag="T3")
            nc.vector.tensor_tensor(out=T3[:], in0=T2[:, 0:DM],
                                    in1=T2[:, DM:2 * DM], op=ALU.add)
            OUTT = spool.tile([128, DM], f32, tag="OUTT")
            nc.vector.tensor_tensor(out=OUTT[:], in0=T3[:],
                                    in1=T1[:, 4 * DM:5 * DM], op=ALU.add)
            nc.sync.dma_start(out=out_d[qs, :], in_=OUTT[:])

    nc.compile()
    return nc


def host_prep(inputs):
    """Build per-core in_maps from full inputs."""
    q = np.ascontiguousarray(inputs["query"], np.float32)
    ref = np.asarray(inputs["reference_points"], np.float32)[..., 0]  # (N,LQ,L)
    vin = np.asarray(inputs["input_flatten"], np.float32)
    W_val = np.asarray(inputs["W_val"], np.float32)
    b_val = np.asarray(inputs["b_val"], np.float32)
    W_off = np.asarray(inputs["W_off"], np.float32)
    b_off = np.asarray(inputs["b_off"], np.float32)
    W_attn = np.asarray(inputs["W_attn"], np.float32)
    b_attn = np.asarray(inputs["b_attn"], np.float32)

    import ml_dtypes
    wv = W_val.T.astype(ml_dtypes.bfloat16)
    wof = np.ascontiguousarray(W_off.T)
    # row 0: b_off - 0.5 ; rows 1..4: SEL[l, c] = T_l * [level(c) == l]
    wofsel = np.zeros((1 + L, M * L * P), np.float32)
    wofsel[0] = b_off - 0.5
    for c in range(M * L * P):
        l = (c % 16) // 4
        wofsel[1 + l, c] = LENS[l]
    wat = np.concatenate([W_attn.T, b_attn[None, :]], 0)

    consts = np.zeros((1, CW), np.float32)
    for l in range(L):
        consts[0, C_TM1L + l] = LENS[l] - 1
        consts[0, C_LST + l] = LSTARTP[l]

    shared = {"wv": np.ascontiguousarray(wv),
              "wvb": np.ascontiguousarray(b_val[None, :]),
              "wof": wof, "wofsel": wofsel,
              "wat": np.ascontiguousarray(wat), "consts": consts}
    return [
        {"qT": np.ascontiguousarray(q[n].T),
         "refT": np.ascontiguousarray(ref[n].T),
         "vinT": np.ascontiguousarray(vin[n].T.astype(ml_dtypes.bfloat16)),
         **shared}
        for n in range(N)
    ]


_NC_CACHE = None


def kernel(**inputs) -> np.ndarray:
    global _NC_CACHE
    if _NC_CACHE is None:
        _NC_CACHE = build_program()
    nc = _NC_CACHE
    in_maps = host_prep(inputs)
    res = run_bass_kernel_spmd(nc, in_maps, list(range(N)))
    return np.stack([res.results[n]["out"] for n in range(N)]).astype(np.float32)


if __name__ == "__main__":
    d = np.load("/root/problem/cached_io.npz")
    inp = {k: d[k] for k in ["query", "reference_points", "input_flatten",
                             "input_temporal_lens", "input_level_start_index",
                             "W_val", "b_val", "W_off", "b_off", "W_attn", "b_attn"]}
    out = kernel(**inp)
    ref = d["ref_out"]
    err = np.abs(out - ref).max()
    print("absmax err:", err, "scale:", np.abs(ref).max(),
          "rel:", err / np.abs(ref).max())
